# revision 9
# baseline (speedup 1.0000x reference)
"""Trainium2 Bass kernel for nn_CustomMixedDecoder (MoE-style mixed decoder).

Math (per batch row b):
    x    = [z, c]                                  # [299]
    coef = softmax(elu(elu(x@gw1+gb1)@gw2+gb2)@gw3+gb3)   # [16]
    h0   = elu(sum_e coef[e] * (x   @ w0[e] + b0[e]))     # [512]
    h1   = elu(sum_e coef[e] * ([z,h0] @ w1[e] + b1[e]))  # [512]
    out  =      sum_e coef[e] * ([z,h1] @ w2[e] + b2[e])  # [267]

Distribution: pure data-parallel over batch. 1024 rows / 8 cores = 128
rows per core == one SBUF partition tile. Expert weights replicated and
host-cast to fp16 to halve the dominating HBM weight traffic (fp32 PSUM
accumulation keeps precision: measured rel err ~5e-4).

Per-core schedule (everything sized so HBM weight streaming is the only
hard resource):
  - host pre-packs [w_l[e]; b_l[e]] into one fp16 array per layer whose
    row count divides evenly into n_kt K-tiles (3x100, 5x109, 5x109 --
    matmul K < 128 costs nothing), so each expert is exactly ONE DMA:
    ~54 DMA instructions total (the serialized HWDGE queue costs ~630 ns
    per DMA instruction).
  - gate MLP computed transposed ([feat, b] layout) in fp32; softmax in
    natural layout after a PE transpose.
  - per expert e, a [128,128] broadcast tile bcast_e[p, b] = coef[b, e]
    is built with one K=16 selector matmul (selector loaded as a host
    constant).
  - per layer: the transposed, ones-augmented input inpT [ks, n_kt, 128]
    is scaled by bcast_e (DVE, fp32 -> fp16) and used as the matmul
    *stationary*; expert weights (bias appended as an extra row to match
    the input's ones-row) stream as the moving operand.  All E * n_kt
    matmuls accumulate into a single PSUM bank, so the expert mixture
    and the bias term come out of PSUM accumulation for free.
  - elu(v) = relu(v) - relu(1 - exp(v)) on the scalar (ACT) engine.
"""

import os
import sys

for _p in ("/opt/trn_rl_repo",):
    if os.path.isdir(_p) and _p not in sys.path:
        sys.path.insert(0, _p)

import numpy as np

import concourse.bacc as bacc
import concourse.mybir as mybir
import concourse.tile as tile
from concourse.bass_utils import run_bass_kernel_spmd
from concourse.masks import make_identity

AFT = mybir.ActivationFunctionType
F32 = mybir.dt.float32
F16 = mybir.dt.float16

NCORES = 8
B = 1024
BS = B // NCORES          # 128 batch rows per core
P = 128
LATENT, FRAME = 32, 267
HID, E, GH = 512, 16, 64
IN_SIZE = LATENT + FRAME  # 299
INTER = LATENT + HID      # 544
OUT_SIZE = FRAME          # 267

# (in_aug, n_kt, ks, out width, has elu) per mixed layer; in_aug counts the
# appended bias/ones row and is chosen to factor exactly as n_kt * ks.
LAYERS = [
    (300, 3, 100, HID, True),
    (545, 5, 109, HID, True),
    (545, 5, 109, OUT_SIZE, False),
]
W_TOTAL_BUFS = 26         # shared weight-tile ring (SBUF budget)


def _emit_body(nc, tc, ctx, d, pools):
    """Emit one full forward pass. d: dict of DRAM APs. pools: tile pools."""
    sing, gate, wp, sp, act, itp, ps_main, ps_sm = (
        pools["sing"], pools["gate"], pools["wp"], pools["sp"],
        pools["act"], pools["itp"], pools["ps_main"], pools["ps_sm"],
    )

    ident = pools["ident"]
    bias0 = pools["bias0"]
    bias1 = pools["bias1"]

    def elu(dst, src, rows, cols):
        t1 = act.tile([P, cols], F32, tag="elu1", bufs=1)
        t2 = act.tile([P, cols], F32, tag="elu2", bufs=1)
        nc.scalar.activation(t1[0:rows, :], src, AFT.Exp, bias=bias0[0:rows, :])
        nc.scalar.activation(t2[0:rows, :], t1[0:rows, :], AFT.Relu,
                             bias=bias1[0:rows, :], scale=-1.0)
        nc.scalar.activation(dst, src, AFT.Relu, bias=bias0[0:rows, :])
        nc.vector.tensor_sub(dst, dst, t2[0:rows, :])

    # ---- x_aug = [z, c, 1] natural; gate weights (fp32, bias row packed) ----
    xa = gate.tile([P, 300], F32, tag="xa")
    nc.sync.dma_start(out=xa[:, 0:IN_SIZE], in_=d["x"][:, :])
    nc.vector.memset(xa[:, 299:300], 1.0)

    gk = LAYERS[0][2]  # 100, gate layer-1 shares layer-0's K tiling
    gw1_sb = gate.tile([gk, 3, GH], F32, tag="gw1")
    nc.sync.dma_start(out=gw1_sb[:, :, :],
                      in_=d["gwa1"].rearrange("(kt p) o -> p kt o", p=gk))
    gw2_sb = gate.tile([GH + 1, GH], F32, tag="gw2")
    nc.gpsimd.dma_start(out=gw2_sb[:, :], in_=d["gwa2"][:, :])
    gw3_sb = gate.tile([GH + 1, E], F32, tag="gw3")
    nc.gpsimd.dma_start(out=gw3_sb[:, :], in_=d["gwa3"][:, :])
    if not pools.get("sel_loaded"):
        nc.gpsimd.dma_start(out=pools["sel"][:, :, :], in_=d["sel"][:, :, :])
        pools["sel_loaded"] = True

    xaT = itp.tile([gk, 3, P], F32, tag="xaT")
    xaT16 = itp.tile([gk, 3, P], F16, tag="xaT16")
    for kt in range(3):
        t_ps = ps_sm.tile([P, P], F32, tag="tp")
        nc.tensor.transpose(t_ps[0:gk, :], xa[:, kt * gk:(kt + 1) * gk],
                            ident[:, :])
        nc.vector.tensor_copy(xaT[:, kt, :], t_ps[0:gk, :])
        nc.vector.tensor_copy(xaT16[:, kt, :], t_ps[0:gk, :])

    # ---- gate MLP (transposed layout) ----
    g1_ps = ps_sm.tile([P, P], F32, tag="g", bufs=1)
    for kt in range(3):
        nc.tensor.matmul(g1_ps[0:GH, :], gw1_sb[:, kt, :], xaT[:, kt, :],
                         start=(kt == 0), stop=(kt == 2))
    h1 = gate.tile([P, P], F32, tag="h1")
    nc.vector.memset(h1[GH:GH + 1, :], 1.0)
    elu(h1[0:GH, :], g1_ps[0:GH, :], GH, P)

    g2_ps = ps_sm.tile([P, P], F32, tag="g", bufs=1)
    nc.tensor.matmul(g2_ps[0:GH, :], gw2_sb[:, :], h1[0:GH + 1, :],
                     start=True, stop=True)
    h2 = gate.tile([P, P], F32, tag="h2")
    nc.vector.memset(h2[GH:GH + 1, :], 1.0)
    elu(h2[0:GH, :], g2_ps[0:GH, :], GH, P)

    g3_ps = ps_sm.tile([P, P], F32, tag="g", bufs=1)
    nc.tensor.matmul(g3_ps[0:E, :], gw3_sb[:, :], h2[0:GH + 1, :],
                     start=True, stop=True)
    lgT = gate.tile([P, P], F32, tag="lgT")
    nc.vector.tensor_copy(lgT[0:E, :], g3_ps[0:E, :])

    # ---- softmax (natural layout) ----
    lg_ps = ps_sm.tile([P, E], F32, tag="g", bufs=1)
    nc.tensor.transpose(lg_ps[:, :], lgT[0:E, :], ident[0:E, 0:E])
    ex = gate.tile([P, E], F32, tag="ex")
    nc.scalar.activation(ex[:, :], lg_ps[:, :], AFT.Exp, bias=bias0[:, :])
    sm = gate.tile([P, 1], F32, tag="sm")
    nc.vector.reduce_sum(sm[:, :], ex[:, :], axis=mybir.AxisListType.X)
    rc = gate.tile([P, 1], F32, tag="rc")
    nc.vector.reciprocal(rc[:, :], sm[:, :])
    coef = gate.tile([P, E], F32, tag="coef")
    nc.vector.tensor_scalar_mul(coef[:, :], ex[:, :], rc[:, :])

    # coef.T then per-expert broadcast tiles bcast_e[p, b] = coef[b, e]
    ct_ps = ps_sm.tile([P, P], F32, tag="g", bufs=1)
    nc.tensor.transpose(ct_ps[0:E, :], coef[:, :], ident[:, :])
    coefT = gate.tile([P, P], F16, tag="coefT")
    nc.vector.tensor_copy(coefT[0:E, :], ct_ps[0:E, :])

    sel = pools["sel"]
    ones16 = pools["ones16"]
    bcast_all = sing.tile([P, E, P], F16, tag="bcast")
    for g in range(4):
        mc = gate.tile([E, 4, P], F16, tag="mc")
        nc.vector.tensor_mul(
            mc[:, :, :], sel[:, g * 4:(g + 1) * 4, :],
            coefT[0:E, :].unsqueeze(1).to_broadcast([E, 4, P]))
        b_ps = ps_sm.tile([P, 4 * P], F32, tag="bc", bufs=3)
        nc.tensor.matmul(b_ps[:, :], ones16[0:E, :],
                         mc[:, :, :].rearrange("k g b -> k (g b)"),
                         start=True, stop=True)
        nc.vector.tensor_copy(
            bcast_all[:, g * 4:(g + 1) * 4, :].rearrange("p g b -> p (g b)"),
            b_ps[:, :])

    # ---- mixed expert layers ----
    inpT = xaT16
    for li, (in_aug, nkt, ks, outw, has_elu) in enumerate(LAYERS):
        w_ap = d[f"wa{li}"]
        pm = ps_main.tile([P, outw], F32, tag="main")
        n_mm = E * nkt
        mm = 0
        for e in range(E):
            w_sb = wp.tile([ks, nkt, outw], F16, tag="w", bufs=W_TOTAL_BUFS)
            nc.sync.dma_start(
                out=w_sb[:, :, :],
                in_=w_ap[e].rearrange("(kt p) o -> p kt o", p=ks))
            sc = sp.tile([ks, nkt, P], F16, tag=f"s{li}")
            nc.vector.tensor_mul(
                sc[:, :, :], inpT[:, :, :],
                bcast_all[0:ks, e, :].unsqueeze(1).to_broadcast([ks, nkt, P]))
            for kt in range(nkt):
                nc.tensor.matmul(pm[:, :], sc[:, kt, :], w_sb[:, kt, :],
                                 start=(mm == 0), stop=(mm == n_mm - 1))
                mm += 1

        if has_elu:
            n_aug, n_nkt, n_ks = (LAYERS[li + 1][0], LAYERS[li + 1][1],
                                  LAYERS[li + 1][2])
            cut = 2 * n_ks
            inat = act.tile([P, n_aug], F32, tag="inat")
            nc.vector.tensor_copy(inat[:, 0:LATENT], xa[:, 0:LATENT])
            # chunk A: inat[:, 0:cut) = [z | elu(pm[:, 0:cut-32])]
            elu(inat[:, LATENT:cut], pm[:, 0:cut - LATENT], P, cut - LATENT)
            # chunk B: inat[:, cut:n_aug) = [elu(pm[:, cut-32:outw]) | 1]
            elu(inat[:, cut:LATENT + outw], pm[:, cut - LATENT:outw], P,
                LATENT + outw - cut)
            nc.vector.memset(inat[:, n_aug - 1:n_aug], 1.0)
            nxt = itp.tile([n_ks, n_nkt, P], F16, tag=f"it{li}")
            for kt in range(n_nkt):
                t_ps = ps_sm.tile([P, P], F32, tag="tp")
                nc.tensor.transpose(t_ps[0:n_ks, :],
                                    inat[:, kt * n_ks:(kt + 1) * n_ks],
                                    ident[:, :])
                nc.vector.tensor_copy(nxt[:, kt, :], t_ps[0:n_ks, :])
            inpT = nxt
        else:
            o_sb = act.tile([P, outw], F32, tag="osb")
            nc.vector.tensor_copy(o_sb[:, :], pm[:, :])
            nc.sync.dma_start(out=d["out"][:, :], in_=o_sb[:, :])


def build_nc(reps=1):
    nc = bacc.Bacc("TRN2", target_bir_lowering=False, debug=False)
    d = {}
    d["x"] = nc.dram_tensor("x", [BS, IN_SIZE], F32, kind="ExternalInput").ap()
    d["gwa1"] = nc.dram_tensor("gwa1", [300, GH], F32, kind="ExternalInput").ap()
    d["gwa2"] = nc.dram_tensor("gwa2", [GH + 1, GH], F32, kind="ExternalInput").ap()
    d["gwa3"] = nc.dram_tensor("gwa3", [GH + 1, E], F32, kind="ExternalInput").ap()
    for li, (in_aug, nkt, ks, outw, _) in enumerate(LAYERS):
        d[f"wa{li}"] = nc.dram_tensor(
            f"wa{li}", [E, in_aug, outw], F16, kind="ExternalInput").ap()
    d["sel"] = nc.dram_tensor("sel", [E, E, P], F16, kind="ExternalInput").ap()
    d["out"] = nc.dram_tensor("out", [BS, OUT_SIZE], F32,
                              kind="ExternalOutput").ap()

    from contextlib import ExitStack
    with tile.TileContext(nc) as tc, ExitStack() as ctx:
        pools = {}
        pools["sing"] = sing = ctx.enter_context(tc.tile_pool(name="sing", bufs=1))
        pools["gate"] = ctx.enter_context(tc.tile_pool(name="gate", bufs=1))
        pools["wp"] = ctx.enter_context(tc.tile_pool(name="wp", bufs=2))
        pools["sp"] = ctx.enter_context(tc.tile_pool(name="sp", bufs=3))
        pools["act"] = ctx.enter_context(tc.tile_pool(name="act", bufs=2))
        pools["itp"] = ctx.enter_context(tc.tile_pool(name="itp", bufs=1))
        pools["ps_main"] = ctx.enter_context(
            tc.tile_pool(name="ps_main", bufs=2, space="PSUM"))
        pools["ps_sm"] = ctx.enter_context(
            tc.tile_pool(name="ps_sm", bufs=2, space="PSUM"))

        ident = sing.tile([P, P], F32, tag="ident")
        make_identity(nc, ident)
        pools["ident"] = ident
        bias0 = sing.tile([P, 1], F32, tag="bias0")
        nc.vector.memset(bias0[:, :], 0.0)
        pools["bias0"] = bias0
        bias1 = sing.tile([P, 1], F32, tag="bias1")
        nc.vector.memset(bias1[:, :], 1.0)
        pools["bias1"] = bias1
        sel = sing.tile([E, E, P], F16, tag="sel")
        pools["sel"] = sel
        pools["sel_loaded"] = False
        ones16 = sing.tile([E, P], F16, tag="ones16")
        nc.vector.memset(ones16[:, :], 1.0)
        pools["ones16"] = ones16

        for _ in range(reps):
            _emit_body(nc, tc, ctx, d, pools)

    nc.compile()
    return nc


_CACHE = {}


def _get_nc(reps=1):
    key = ("nc", reps)
    if key not in _CACHE:
        _CACHE[key] = build_nc(reps)
    return _CACHE[key]


def make_in_maps(inputs):
    z = np.asarray(inputs["z"], dtype=np.float32)
    c = np.asarray(inputs["c"], dtype=np.float32)
    x = np.ascontiguousarray(np.concatenate([z, c], axis=1))

    rep = {}
    rep["gwa1"] = np.ascontiguousarray(np.concatenate(
        [np.asarray(inputs["gw1"], np.float32),
         np.asarray(inputs["gb1"], np.float32)[None, :]], axis=0))
    rep["gwa2"] = np.ascontiguousarray(np.concatenate(
        [np.asarray(inputs["gw2"], np.float32),
         np.asarray(inputs["gb2"], np.float32)[None, :]], axis=0))
    rep["gwa3"] = np.ascontiguousarray(np.concatenate(
        [np.asarray(inputs["gw3"], np.float32),
         np.asarray(inputs["gb3"], np.float32)[None, :]], axis=0))
    for li, (wk, bk) in enumerate((("w0", "b0"), ("w1", "b1"), ("w2", "b2"))):
        w = np.asarray(inputs[wk]).astype(np.float16)
        b = np.asarray(inputs[bk]).astype(np.float16)
        rep[f"wa{li}"] = np.ascontiguousarray(
            np.concatenate([w, b[:, None, :]], axis=1))
    sel = np.zeros((E, E, P), np.float16)
    for e in range(E):
        sel[e, e, :] = 1.0
    rep["sel"] = sel

    in_maps = []
    for i in range(NCORES):
        m = {"x": x[i * BS:(i + 1) * BS]}
        m.update(rep)
        in_maps.append(m)
    return in_maps


def kernel(**inputs):
    nc = _get_nc(reps=1)
    in_maps = make_in_maps(inputs)
    res = run_bass_kernel_spmd(nc, in_maps, list(range(NCORES)))
    return np.concatenate([res.results[i]["out"] for i in range(NCORES)],
                          axis=0)


if __name__ == "__main__":
    rng = np.random.default_rng(0)
    ins = {
        "z": rng.standard_normal((B, LATENT), dtype=np.float32),
        "c": rng.standard_normal((B, FRAME), dtype=np.float32),
        "gw1": rng.standard_normal((IN_SIZE, GH), dtype=np.float32) / 17.3,
        "gb1": np.zeros(GH, np.float32),
        "gw2": rng.standard_normal((GH, GH), dtype=np.float32) / 8.0,
        "gb2": np.zeros(GH, np.float32),
        "gw3": rng.standard_normal((GH, E), dtype=np.float32) / 8.0,
        "gb3": np.zeros(E, np.float32),
        "w0": rng.standard_normal((E, IN_SIZE, HID), dtype=np.float32) / 17.3,
        "b0": np.full((E, HID), 0.01, np.float32),
        "w1": rng.standard_normal((E, INTER, HID), dtype=np.float32) / 23.3,
        "b1": np.full((E, HID), 0.01, np.float32),
        "w2": rng.standard_normal((E, INTER, OUT_SIZE), dtype=np.float32) / 23.3,
        "b2": np.full((E, OUT_SIZE), 0.01, np.float32),
    }
    out = kernel(**ins)
    print("kernel out", out.shape, out.dtype, np.abs(out).max())


# revision 12
# speedup vs baseline: 562.8041x; 562.8041x over previous
"""Trainium2 Bass kernel for nn_CustomMixedDecoder (MoE-style mixed decoder).

Math (per batch row b):
    x    = [z, c]                                  # [299]
    coef = softmax(elu(elu(x@gw1+gb1)@gw2+gb2)@gw3+gb3)   # [16]
    h0   = elu(sum_e coef[e] * (x   @ w0[e] + b0[e]))     # [512]
    h1   = elu(sum_e coef[e] * ([z,h0] @ w1[e] + b1[e]))  # [512]
    out  =      sum_e coef[e] * ([z,h1] @ w2[e] + b2[e])  # [267]

Distribution: pure data-parallel over batch. 1024 rows / 8 cores = 128
rows per core == one SBUF partition tile. Expert weights replicated and
host-cast to fp16 to halve the dominating HBM weight traffic (fp32 PSUM
accumulation keeps precision: measured rel err ~5e-4).

Per-core schedule (everything sized so HBM weight streaming is the only
hard resource):
  - host pre-packs [w_l[e]; b_l[e]] into one fp16 array per layer whose
    row count divides evenly into n_kt K-tiles (3x100, 5x109, 5x109 --
    matmul K < 128 costs nothing), so each expert is exactly ONE DMA:
    ~54 DMA instructions total (the serialized HWDGE queue costs ~630 ns
    per DMA instruction).
  - gate MLP computed transposed ([feat, b] layout) in fp32; softmax in
    natural layout after a PE transpose.
  - per expert e, a [128,128] broadcast tile bcast_e[p, b] = coef[b, e]
    is built with one K=16 selector matmul (selector loaded as a host
    constant).
  - per layer: the transposed, ones-augmented input inpT [ks, n_kt, 128]
    is scaled by bcast_e (DVE, fp32 -> fp16) and used as the matmul
    *stationary*; expert weights (bias appended as an extra row to match
    the input's ones-row) stream as the moving operand.  All E * n_kt
    matmuls accumulate into a single PSUM bank, so the expert mixture
    and the bias term come out of PSUM accumulation for free.
  - elu(v) = relu(v) - relu(1 - exp(v)) on the scalar (ACT) engine.
"""

import os
import sys

for _p in ("/opt/trn_rl_repo",):
    if os.path.isdir(_p) and _p not in sys.path:
        sys.path.insert(0, _p)

import numpy as np

import concourse.bacc as bacc
import concourse.mybir as mybir
import concourse.tile as tile
from concourse.bass_utils import run_bass_kernel_spmd
from concourse.masks import make_identity

AFT = mybir.ActivationFunctionType
F32 = mybir.dt.float32
F16 = mybir.dt.float16

NCORES = 8
B = 1024
BS = B // NCORES          # 128 batch rows per core
P = 128
LATENT, FRAME = 32, 267
HID, E, GH = 512, 16, 64
IN_SIZE = LATENT + FRAME  # 299
INTER = LATENT + HID      # 544
OUT_SIZE = FRAME          # 267

# (in_aug, n_kt, ks, out width, has elu) per mixed layer; in_aug counts the
# appended bias/ones row and is chosen to factor exactly as n_kt * ks.
LAYERS = [
    (300, 3, 100, HID, True),
    (545, 5, 109, HID, True),
    (545, 5, 109, OUT_SIZE, False),
]
W_TOTAL_BUFS = 28         # shared weight-tile ring (SBUF budget)


def _emit_body(nc, tc, ctx, d, pools):
    """Emit one full forward pass. d: dict of DRAM APs. pools: tile pools."""
    sing, gate, wp, sp, act, itp, ps_main, ps_sm = (
        pools["sing"], pools["gate"], pools["wp"], pools["sp"],
        pools["act"], pools["itp"], pools["ps_main"], pools["ps_sm"],
    )

    ident = pools["ident"]
    bias0 = pools["bias0"]
    bias1 = pools["bias1"]

    def elu(dst, src, rows, cols, scale=None):
        tm = act.tile([P, cols], F32, tag="elu1", bufs=1)
        te = act.tile([P, cols], F32, tag="elu2", bufs=1)
        tr = act.tile([P, cols], F32, tag="elu3", bufs=1)
        if scale is None:
            nc.vector.tensor_scalar_min(tm[0:rows, :], src, 0.0)
            nc.scalar.activation(tr[0:rows, :], src, AFT.Relu,
                                 bias=bias0[0:rows, :])
        else:
            nc.vector.tensor_scalar(tm[0:rows, :], src, scale, 0.0,
                                    op0=mybir.AluOpType.mult,
                                    op1=mybir.AluOpType.min)
            nc.scalar.activation(tr[0:rows, :], src, AFT.Relu,
                                 bias=bias0[0:rows, :], scale=scale)
        nc.scalar.activation(te[0:rows, :], tm[0:rows, :], AFT.Exp,
                             bias=bias0[0:rows, :])
        nc.vector.scalar_tensor_tensor(dst, te[0:rows, :], -1.0, tr[0:rows, :],
                                       mybir.AluOpType.add, mybir.AluOpType.add)

    # ---- x_aug = [z, c, 1] natural; gate weights (fp32, bias row packed) ----
    xa = gate.tile([P, 300], F32, tag="xa")
    nc.sync.dma_start(out=xa[:, 0:IN_SIZE], in_=d["x"][:, :])
    nc.vector.memset(xa[:, 299:300], 1.0)

    gk = LAYERS[0][2]  # 100, gate layer-1 shares layer-0's K tiling
    gw1_sb = gate.tile([gk, 3, GH], F32, tag="gw1")
    nc.sync.dma_start(out=gw1_sb[:, :, :],
                      in_=d["gwa1"].rearrange("(kt p) o -> p kt o", p=gk))
    gw2_sb = gate.tile([GH + 1, GH], F32, tag="gw2")
    nc.gpsimd.dma_start(out=gw2_sb[:, :], in_=d["gwa2"][:, :])
    gw3_sb = gate.tile([GH + 1, E], F32, tag="gw3")
    nc.gpsimd.dma_start(out=gw3_sb[:, :], in_=d["gwa3"][:, :])
    if not pools.get("sel_loaded"):
        nc.gpsimd.dma_start(out=pools["sel"][:, :, :], in_=d["sel"][:, :, :])
        pools["sel_loaded"] = True

    xaT = itp.tile([gk, 3, P], F32, tag="xaT")
    xaT16 = itp.tile([gk, 3, P], F16, tag="xaT16")
    for kt in range(3):
        t_ps = ps_sm.tile([P, P], F32, tag="tp")
        nc.tensor.transpose(t_ps[0:gk, :], xa[:, kt * gk:(kt + 1) * gk],
                            ident[:, :])
        nc.vector.tensor_copy(xaT[:, kt, :], t_ps[0:gk, :])
        nc.vector.tensor_copy(xaT16[:, kt, :], t_ps[0:gk, :])

    # ---- gate MLP (transposed layout) ----
    g1_ps = ps_sm.tile([P, P], F32, tag="g", bufs=1)
    for kt in range(3):
        nc.tensor.matmul(g1_ps[0:GH, :], gw1_sb[:, kt, :], xaT[:, kt, :],
                         start=(kt == 0), stop=(kt == 2))
    h1 = gate.tile([P, P], F32, tag="h1")
    nc.vector.memset(h1[GH:GH + 1, :], 1.0)
    elu(h1[0:GH, :], g1_ps[0:GH, :], GH, P)

    g2_ps = ps_sm.tile([P, P], F32, tag="g", bufs=1)
    nc.tensor.matmul(g2_ps[0:GH, :], gw2_sb[:, :], h1[0:GH + 1, :],
                     start=True, stop=True)
    h2 = gate.tile([P, P], F32, tag="h2")
    nc.vector.memset(h2[GH:GH + 1, :], 1.0)
    elu(h2[0:GH, :], g2_ps[0:GH, :], GH, P)

    g3_ps = ps_sm.tile([P, P], F32, tag="g", bufs=1)
    nc.tensor.matmul(g3_ps[0:E, :], gw3_sb[:, :], h2[0:GH + 1, :],
                     start=True, stop=True)

    # unnormalized transposed softmax: exT[e, b] = exp(logit[b, e]); the
    # 1/sum(exp) factor is folded into each layer's output scale (rc).
    exT = gate.tile([E, P], F16, tag="exT")
    nc.scalar.activation(exT[:, :], g3_ps[0:E, :], AFT.Exp, bias=bias0[0:E, :])

    sel = pools["sel"]
    ones16 = pools["ones16"]
    bcast_all = sing.tile([P, E, P], F16, tag="bcast")
    for g in range(4):
        mc = gate.tile([E, 4, P], F16, tag="mc")
        nc.vector.tensor_mul(
            mc[:, :, :], sel[:, g * 4:(g + 1) * 4, :],
            exT[:, :].unsqueeze(1).to_broadcast([E, 4, P]))
        b_ps = ps_sm.tile([P, 4 * P], F32, tag="bc", bufs=3)
        nc.tensor.matmul(b_ps[:, :], ones16[0:E, :],
                         mc[:, :, :].rearrange("k g b -> k (g b)"),
                         start=True, stop=True)
        nc.vector.tensor_copy(
            bcast_all[:, g * 4:(g + 1) * 4, :].rearrange("p g b -> p (g b)"),
            b_ps[:, :])

    # rc[b] = 1 / sum_e exp(logit[b, e])  (off the critical path: first
    # needed at layer 0's epilogue)
    s_ps = ps_sm.tile([1, P], F32, tag="g", bufs=1)
    nc.tensor.matmul(s_ps[:, :], ones16[0:E, 0:1], exT[:, :],
                     start=True, stop=True)
    s_sb = gate.tile([1, P], F32, tag="ssb")
    nc.vector.tensor_copy(s_sb[:, :], s_ps[:, :])
    r_sb = gate.tile([1, P], F32, tag="rsb")
    nc.vector.reciprocal(r_sb[:, :], s_sb[:, :])
    rT_ps = ps_sm.tile([P, 1], F32, tag="g", bufs=1)
    nc.tensor.transpose(rT_ps[:, :], r_sb[:, :], ident[0:1, 0:1])
    rc = gate.tile([P, 1], F32, tag="rc")
    nc.vector.tensor_copy(rc[:, :], rT_ps[:, :])

    # ---- mixed expert layers ----
    inpT = xaT16
    for li, (in_aug, nkt, ks, outw, has_elu) in enumerate(LAYERS):
        w_ap = d[f"wa{li}"]
        pm = ps_main.tile([P, outw], F32, tag="main")
        n_mm = E * nkt
        mm = 0
        for e in range(E):
            w_sb = wp.tile([ks, nkt, outw], F16, tag="w", bufs=W_TOTAL_BUFS)
            nc.sync.dma_start(
                out=w_sb[:, :, :],
                in_=w_ap[e].rearrange("(kt p) o -> p kt o", p=ks))
            sc = sp.tile([ks, nkt, P], F16, tag=f"s{li}")
            if e == 0 and nkt == 5:
                nc.vector.tensor_mul(
                    sc[:, 0:2, :], inpT[:, 0:2, :],
                    bcast_all[0:ks, e, :].unsqueeze(1).to_broadcast([ks, 2, P]))
                nc.vector.tensor_mul(
                    sc[:, 2:5, :], inpT[:, 2:5, :],
                    bcast_all[0:ks, e, :].unsqueeze(1).to_broadcast([ks, 3, P]))
            else:
                nc.vector.tensor_mul(
                    sc[:, :, :], inpT[:, :, :],
                    bcast_all[0:ks, e, :].unsqueeze(1).to_broadcast([ks, nkt, P]))
            for kt in range(nkt):
                nc.tensor.matmul(pm[:, :], sc[:, kt, :], w_sb[:, kt, :],
                                 start=(mm == 0), stop=(mm == n_mm - 1))
                mm += 1

        if has_elu:
            n_aug, n_nkt, n_ks = (LAYERS[li + 1][0], LAYERS[li + 1][1],
                                  LAYERS[li + 1][2])
            cut = 2 * n_ks
            inat = act.tile([P, n_aug], F32, tag="inat")
            nc.vector.tensor_copy(inat[:, 0:LATENT], xa[:, 0:LATENT])
            # chunk A: inat[:, 0:cut) = [z | elu(pm[:, 0:cut-32])]
            elu(inat[:, LATENT:cut], pm[:, 0:cut - LATENT], P, cut - LATENT,
                scale=rc[:, :])
            # chunk B: inat[:, cut:n_aug) = [elu(pm[:, cut-32:outw]) | 1]
            elu(inat[:, cut:LATENT + outw], pm[:, cut - LATENT:outw], P,
                LATENT + outw - cut, scale=rc[:, :])
            nc.vector.memset(inat[:, n_aug - 1:n_aug], 1.0)
            nxt = itp.tile([n_ks, n_nkt, P], F16, tag=f"it{li}")
            for kt in range(n_nkt):
                t_ps = ps_sm.tile([P, P], F32, tag="tp")
                nc.tensor.transpose(t_ps[0:n_ks, :],
                                    inat[:, kt * n_ks:(kt + 1) * n_ks],
                                    ident[:, :])
                nc.vector.tensor_copy(nxt[:, kt, :], t_ps[0:n_ks, :])
            inpT = nxt
        else:
            o_sb = act.tile([P, outw], F32, tag="osb")
            half = outw // 2
            for lo, hi in ((0, half), (half, outw)):
                nc.vector.tensor_scalar_mul(o_sb[:, lo:hi], pm[:, lo:hi],
                                            rc[:, :])
                nc.sync.dma_start(out=d["out"][:, lo:hi], in_=o_sb[:, lo:hi])


def build_nc(reps=1):
    nc = bacc.Bacc("TRN2", target_bir_lowering=False, debug=False)
    d = {}
    d["x"] = nc.dram_tensor("x", [BS, IN_SIZE], F32, kind="ExternalInput").ap()
    d["gwa1"] = nc.dram_tensor("gwa1", [300, GH], F32, kind="ExternalInput").ap()
    d["gwa2"] = nc.dram_tensor("gwa2", [GH + 1, GH], F32, kind="ExternalInput").ap()
    d["gwa3"] = nc.dram_tensor("gwa3", [GH + 1, E], F32, kind="ExternalInput").ap()
    for li, (in_aug, nkt, ks, outw, _) in enumerate(LAYERS):
        d[f"wa{li}"] = nc.dram_tensor(
            f"wa{li}", [E, in_aug, outw], F16, kind="ExternalInput").ap()
    d["sel"] = nc.dram_tensor("sel", [E, E, P], F16, kind="ExternalInput").ap()
    d["out"] = nc.dram_tensor("out", [BS, OUT_SIZE], F32,
                              kind="ExternalOutput").ap()

    from contextlib import ExitStack
    with tile.TileContext(nc) as tc, ExitStack() as ctx:
        pools = {}
        pools["sing"] = sing = ctx.enter_context(tc.tile_pool(name="sing", bufs=1))
        pools["gate"] = ctx.enter_context(tc.tile_pool(name="gate", bufs=1))
        pools["wp"] = ctx.enter_context(tc.tile_pool(name="wp", bufs=2))
        pools["sp"] = ctx.enter_context(tc.tile_pool(name="sp", bufs=4))
        pools["act"] = ctx.enter_context(tc.tile_pool(name="act", bufs=2))
        pools["itp"] = ctx.enter_context(tc.tile_pool(name="itp", bufs=1))
        pools["ps_main"] = ctx.enter_context(
            tc.tile_pool(name="ps_main", bufs=2, space="PSUM"))
        pools["ps_sm"] = ctx.enter_context(
            tc.tile_pool(name="ps_sm", bufs=2, space="PSUM"))

        ident = sing.tile([P, P], F32, tag="ident")
        make_identity(nc, ident)
        pools["ident"] = ident
        bias0 = sing.tile([P, 1], F32, tag="bias0")
        nc.vector.memset(bias0[:, :], 0.0)
        pools["bias0"] = bias0
        bias1 = sing.tile([P, 1], F32, tag="bias1")
        nc.vector.memset(bias1[:, :], 1.0)
        pools["bias1"] = bias1
        sel = sing.tile([E, E, P], F16, tag="sel")
        pools["sel"] = sel
        pools["sel_loaded"] = False
        ones16 = sing.tile([E, P], F16, tag="ones16")
        nc.vector.memset(ones16[:, :], 1.0)
        pools["ones16"] = ones16

        for _ in range(reps):
            _emit_body(nc, tc, ctx, d, pools)

    nc.compile()
    return nc


_CACHE = {}


def _get_nc(reps=1):
    key = ("nc", reps)
    if key not in _CACHE:
        _CACHE[key] = build_nc(reps)
    return _CACHE[key]


def make_in_maps(inputs):
    z = np.asarray(inputs["z"], dtype=np.float32)
    c = np.asarray(inputs["c"], dtype=np.float32)
    x = np.ascontiguousarray(np.concatenate([z, c], axis=1))

    rep = {}
    rep["gwa1"] = np.ascontiguousarray(np.concatenate(
        [np.asarray(inputs["gw1"], np.float32),
         np.asarray(inputs["gb1"], np.float32)[None, :]], axis=0))
    rep["gwa2"] = np.ascontiguousarray(np.concatenate(
        [np.asarray(inputs["gw2"], np.float32),
         np.asarray(inputs["gb2"], np.float32)[None, :]], axis=0))
    rep["gwa3"] = np.ascontiguousarray(np.concatenate(
        [np.asarray(inputs["gw3"], np.float32),
         np.asarray(inputs["gb3"], np.float32)[None, :]], axis=0))
    for li, (wk, bk) in enumerate((("w0", "b0"), ("w1", "b1"), ("w2", "b2"))):
        w = np.asarray(inputs[wk]).astype(np.float16)
        b = np.asarray(inputs[bk]).astype(np.float16)
        rep[f"wa{li}"] = np.ascontiguousarray(
            np.concatenate([w, b[:, None, :]], axis=1))
    sel = np.zeros((E, E, P), np.float16)
    for e in range(E):
        sel[e, e, :] = 1.0
    rep["sel"] = sel

    in_maps = []
    for i in range(NCORES):
        m = {"x": x[i * BS:(i + 1) * BS]}
        m.update(rep)
        in_maps.append(m)
    return in_maps


def kernel(**inputs):
    nc = _get_nc(reps=1)
    in_maps = make_in_maps(inputs)
    res = run_bass_kernel_spmd(nc, in_maps, list(range(NCORES)))
    return np.concatenate([res.results[i]["out"] for i in range(NCORES)],
                          axis=0)


if __name__ == "__main__":
    rng = np.random.default_rng(0)
    ins = {
        "z": rng.standard_normal((B, LATENT), dtype=np.float32),
        "c": rng.standard_normal((B, FRAME), dtype=np.float32),
        "gw1": rng.standard_normal((IN_SIZE, GH), dtype=np.float32) / 17.3,
        "gb1": np.zeros(GH, np.float32),
        "gw2": rng.standard_normal((GH, GH), dtype=np.float32) / 8.0,
        "gb2": np.zeros(GH, np.float32),
        "gw3": rng.standard_normal((GH, E), dtype=np.float32) / 8.0,
        "gb3": np.zeros(E, np.float32),
        "w0": rng.standard_normal((E, IN_SIZE, HID), dtype=np.float32) / 17.3,
        "b0": np.full((E, HID), 0.01, np.float32),
        "w1": rng.standard_normal((E, INTER, HID), dtype=np.float32) / 23.3,
        "b1": np.full((E, HID), 0.01, np.float32),
        "w2": rng.standard_normal((E, INTER, OUT_SIZE), dtype=np.float32) / 23.3,
        "b2": np.full((E, OUT_SIZE), 0.01, np.float32),
    }
    out = kernel(**ins)
    print("kernel out", out.shape, out.dtype, np.abs(out).max())
